# revision 1
# baseline (speedup 1.0000x reference)
"""Trainium2 Bass kernel for nn_AttnDecoder_87230785782556.

Multi-head attention decoder layer: out = softmax((xq Wq)(xk Wk)^T * s) (xv Wv) Wo
Sharding: 8 cores = 2 batches x 4 head-groups (4 heads each, tensor-parallel
column split of Wq/Wk/Wv, row split of Wo; partial outputs summed on host).

All matmuls run in float32r (full PE rate, ~1.5e-4 rounding). Scores are
computed transposed [kv, q] so exp output feeds attn@V as the moving operand;
an extra all-ones column in the V weights makes the same matmul emit the
softmax denominator. Normalization uses an indicator-matmul to broadcast
1/den across partitions.
"""
import math
import numpy as np

from concourse import bacc, mybir, tile
from concourse.bass_utils import run_bass_kernel_spmd

# Problem constants (hardcoded per contract)
B = 2
SEQ = 2048
E = 1024
NUM_HEADS = 16
HD = 64
QK_SCALE = 0.125
N_CORES = 8
HPC = 4            # heads per core
P = 128
NQ = 512           # q chunk (matmul moving free dim)

F32 = mybir.dt.float32
F32R = mybir.dt.float32r


def build_program(seq=SEQ, repeat=1):
    """Build the per-core SPMD program. Identical on all 8 cores."""
    nc = bacc.Bacc("TRN2", target_bir_lowering=False, debug=False,
                   num_devices=N_CORES)

    n_qc = seq // NQ            # q chunks
    n_kv = seq // P             # kv tiles of 128
    n_kt = E // P               # embedding contraction tiles
    FPC = HPC * HD              # features per core = 256
    n_m = FPC // P              # feature pair-tiles = 2

    xtq = nc.dram_tensor("xtq", [E, seq], F32R, kind="ExternalInput")
    xtk = nc.dram_tensor("xtk", [E, seq], F32R, kind="ExternalInput")
    xtv = nc.dram_tensor("xtv", [E, seq], F32R, kind="ExternalInput")
    wq = nc.dram_tensor("wq", [E, FPC], F32R, kind="ExternalInput")
    wk = nc.dram_tensor("wk", [E, FPC], F32R, kind="ExternalInput")
    wv = nc.dram_tensor("wv", [E, HPC * (HD + 1)], F32R, kind="ExternalInput")
    wo = nc.dram_tensor("wo", [FPC, E], F32R, kind="ExternalInput")
    ind = nc.dram_tensor("ind", [HPC, n_m, P], F32R, kind="ExternalInput")
    out = nc.dram_tensor("out", [seq, E], F32, kind="ExternalOutput")

    with tile.TileContext(nc) as tc, nc.allow_low_precision("f32r pipeline"):
        import contextlib
        ctx = contextlib.ExitStack()
        with ctx:
            consts = ctx.enter_context(tc.tile_pool(name="consts", bufs=1))
            bigs = ctx.enter_context(tc.tile_pool(name="bigs", bufs=1))
            vhp = ctx.enter_context(tc.tile_pool(name="vhp", bufs=n_kv))
            xs = ctx.enter_context(tc.tile_pool(name="xs", bufs=6))
            expp = ctx.enter_context(tc.tile_pool(name="expp", bufs=11))
            stkp = ctx.enter_context(tc.tile_pool(name="stkp", bufs=4))
            denp = ctx.enter_context(tc.tile_pool(name="denp", bufs=2))
            finp = ctx.enter_context(tc.tile_pool(name="finp", bufs=2))
            ps = ctx.enter_context(tc.tile_pool(name="ps", bufs=4, space="PSUM"))

            # ---- resident constants ----
            wq_t = consts.tile([P, n_kt, FPC], F32R, name="wq_t", tag="wq")
            wk_t = consts.tile([P, n_kt, FPC], F32R, name="wk_t", tag="wk")
            wv_t = consts.tile([P, n_kt, HPC * (HD + 1)], F32R, name="wv_t", tag="wv")
            wo_t = consts.tile([P, n_m, E], F32R, name="wo_t", tag="wo")
            ind_t = consts.tile([HPC, n_m, P], F32R, name="ind_t", tag="ind")
            nc.sync.dma_start(out=wq_t, in_=wq.ap().rearrange("(t p) m -> p t m", p=P))
            nc.sync.dma_start(out=wk_t, in_=wk.ap().rearrange("(t p) m -> p t m", p=P))
            nc.sync.dma_start(out=wv_t, in_=wv.ap().rearrange("(t p) m -> p t m", p=P))
            nc.sync.dma_start(out=wo_t, in_=wo.ap().rearrange("(t p) m -> p t m", p=P))
            nc.sync.dma_start(out=ind_t, in_=ind.ap())

            for rep in range(repeat):
                # ---- K projection: khT[m] [128, seq], partitions = pair feats ----
                khT = [bigs.tile([P, seq], F32R, name=f"khT{m}", tag=f"khT{m}") for m in range(n_m)]
                qhT = [bigs.tile([P, seq], F32R, name=f"qhT{m}", tag=f"qhT{m}") for m in range(n_m)]
                for nq in range(n_qc):
                    ps_t = ps.tile([P, 2, NQ], F32, name="ps_t", tag="ps")
                    for kt in range(n_kt):
                        x_t = xs.tile([P, NQ], F32R, name="xk_t", tag="x")
                        nc.sync.dma_start(
                            out=x_t,
                            in_=xtk.ap()[P * kt:P * (kt + 1), NQ * nq:NQ * (nq + 1)])
                        for m in range(n_m):
                            nc.tensor.matmul(
                                ps_t[:, m, :],
                                wk_t[:, kt, P * m:P * (m + 1)],
                                x_t,
                                start=(kt == 0), stop=(kt == n_kt - 1))
                    for m in range(n_m):
                        nc.vector.tensor_copy(
                            khT[m][:, NQ * nq:NQ * (nq + 1)], ps_t[:, m, :])

                # ---- V projection: vh tiles [128, 4, 65] (65th col -> ones) ----
                vh_tiles = [vhp.tile([P, HPC, HD + 1], F32R, name=f"vh{i}", tag="vh")
                            for i in range(n_kv)]
                for mb in range(n_kv // 4):
                    ps_v = [ps.tile([P, 2, NQ], F32, name="ps_t", tag="ps") for _ in range(2)]
                    for kt in range(n_kt):
                        xv_t = xs.tile([P, NQ], F32R, name="xv_t", tag="x")
                        nc.sync.dma_start(
                            out=xv_t,
                            in_=xtv.ap()[P * kt:P * (kt + 1), NQ * mb:NQ * (mb + 1)])
                        for sub in range(4):
                            nc.tensor.matmul(
                                ps_v[sub // 2][:, sub % 2, 0:HPC * (HD + 1)],
                                xv_t[:, P * sub:P * (sub + 1)],
                                wv_t[:, kt, :],
                                start=(kt == 0), stop=(kt == n_kt - 1))
                    for sub in range(4):
                        mk = 4 * mb + sub
                        nc.vector.tensor_copy(
                            vh_tiles[mk],
                            ps_v[sub // 2][:, sub % 2, 0:HPC * (HD + 1)].rearrange(
                                "p (h c) -> p h c", h=HPC))
                        # ones slots (zero in wv): even heads col 64, odd heads col 0
                        nc.vector.tensor_scalar_add(
                            vh_tiles[mk][:, 0::2, HD], vh_tiles[mk][:, 0::2, HD], 1.0)
                        nc.vector.tensor_scalar_add(
                            vh_tiles[mk][:, 1::2, 0], vh_tiles[mk][:, 1::2, 0], 1.0)

                # ---- Q projection (per chunk) + attention + output projection ----
                def emit_norm_outproj(stk_tiles, den_t, qc):
                    # normalize: bcast 1/den across partitions via indicator matmul
                    rcp_t = denp.tile([HPC, NQ], F32R, name="rcp_t", tag="rcp")
                    nc.vector.reciprocal(rcp_t, den_t)
                    for pair in range(n_m):
                        bc_ps = ps.tile([P, 2, NQ], F32, name="ps_t", tag="ps")
                        nc.tensor.matmul(bc_ps[:, 0, :], ind_t[:, pair, :], rcp_t,
                                         start=True, stop=True)
                        nc.vector.tensor_mul(stk_tiles[pair], stk_tiles[pair],
                                             bc_ps[:, 0, :])
                    # output projection: out[q, E] = sum_pair stk[pair].T @ wo[pair]
                    for qs in range(NQ // P):
                        op_ps = ps.tile([P, 2, NQ], F32, name="ps_t", tag="ps")
                        for nch in range(E // NQ):
                            for pair in range(n_m):
                                nc.tensor.matmul(
                                    op_ps[:, nch, :],
                                    stk_tiles[pair][:, P * qs:P * (qs + 1)],
                                    wo_t[:, pair, NQ * nch:NQ * (nch + 1)],
                                    start=(pair == 0), stop=(pair == n_m - 1))
                        fin_t = finp.tile([P, 2, NQ], F32, name="fin_t", tag="fin")
                        for nch in range(E // NQ):
                            nc.vector.tensor_copy(fin_t[:, nch, :], op_ps[:, nch, :])
                        r0 = NQ * qc + P * qs
                        nc.sync.dma_start(
                            out=out.ap()[r0:r0 + P, :],
                            in_=fin_t.rearrange("p a b -> p (a b)"))

                pending = None
                for qc in range(n_qc):
                    ps_q = ps.tile([P, 2, NQ], F32, name="ps_t", tag="ps")
                    for kt in range(n_kt):
                        xq_t = xs.tile([P, NQ], F32R, name="xq_t", tag="x")
                        nc.sync.dma_start(
                            out=xq_t,
                            in_=xtq.ap()[P * kt:P * (kt + 1), NQ * qc:NQ * (qc + 1)])
                        for m in range(n_m):
                            nc.tensor.matmul(
                                ps_q[:, m, :],
                                wq_t[:, kt, P * m:P * (m + 1)],
                                xq_t,
                                start=(kt == 0), stop=(kt == n_kt - 1))
                    for m in range(n_m):
                        nc.vector.tensor_copy(
                            qhT[m][:, NQ * qc:NQ * (qc + 1)], ps_q[:, m, :])

                    den_t = denp.tile([HPC, NQ], F32R, name="den_t", tag="den")
                    stk_tiles = []
                    for pair in range(n_m):
                        att_ps = ps.tile([P, 2, NQ], F32, name="ps_t", tag="ps")
                        exp_tiles = []

                        def emit_attnv(g, pair=pair, att_ps=att_ps,
                                       exp_tiles=exp_tiles):
                            for s in range(2):
                                kv = 2 * g + s
                                for h01 in range(2):
                                    nc.tensor.matmul(
                                        att_ps[0:HD + 1, h01, :],
                                        vh_tiles[kv][:, 2 * pair + h01, :],
                                        exp_tiles[g][h01][:, s, :],
                                        start=(kv == 0), stop=(kv == n_kv - 1))

                        # software pipeline: attn@V for group g-1 interleaves with
                        # scores+exp for group g, so exp tiles retire promptly and
                        # the PE never queues behind a not-yet-runnable exp.
                        for g in range(n_kv // 2):
                            sc = []
                            for h01 in range(2):
                                sc_t = ps.tile([P, 2, NQ], F32, name="ps_t", tag="ps")
                                for s in range(2):
                                    kv = 2 * g + s
                                    nc.tensor.matmul(
                                        sc_t[:, s, :],
                                        khT[pair][64 * h01:64 * (h01 + 1),
                                                  P * kv:P * (kv + 1)],
                                        qhT[pair][64 * h01:64 * (h01 + 1),
                                                  NQ * qc:NQ * (qc + 1)],
                                        start=True, stop=True,
                                        tile_position=(64 * h01, 0))
                                sc.append(sc_t)
                            pair_exp = []
                            for h01 in range(2):
                                e_t = expp.tile([P, 2, NQ], F32R, name="exp_t", tag="exp")
                                nc.scalar.activation(
                                    e_t, sc[h01],
                                    mybir.ActivationFunctionType.Exp)
                                pair_exp.append(e_t)
                            exp_tiles.append(pair_exp)
                            if g > 0:
                                emit_attnv(g - 1)
                        emit_attnv(n_kv // 2 - 1)
                        # stack the two heads' outputs [feat, q] into one tile.
                        # PSUM is not DMA-accessible: stage through SBUF, then
                        # SBUF->SBUF DMAs do the cross-partition moves.
                        stk_t = stkp.tile([P, NQ], F32R, name="stk_t", tag="stk")
                        nc.vector.tensor_copy(stk_t[0:HD, :], att_ps[0:HD, 0, :])
                        sodd_t = stkp.tile([P, NQ], F32R, name="sodd_t", tag="sodd", bufs=2)
                        nc.vector.tensor_copy(sodd_t[0:HD + 1, :],
                                              att_ps[0:HD + 1, 1, :])
                        dstg_t = stkp.tile([P, NQ], F32R, name="dstg_t", tag="dstg", bufs=2)
                        nc.vector.tensor_copy(dstg_t[HD:HD + 1, :],
                                              att_ps[HD:HD + 1, 0, :])
                        nc.sync.dma_start(out=stk_t[HD:P, :], in_=sodd_t[1:HD + 1, :])
                        # denominators: even head at psum row 64 slot 0, odd at row 0 slot 1
                        nc.sync.dma_start(out=den_t[2 * pair:2 * pair + 1, :],
                                          in_=dstg_t[HD:HD + 1, :])
                        nc.sync.dma_start(out=den_t[2 * pair + 1:2 * pair + 2, :],
                                          in_=sodd_t[0:1, :])
                        stk_tiles.append(stk_t)

                    # defer normalization + output projection by one chunk so the
                    # den DMA/recip chain overlaps the next chunk's attention.
                    if pending is not None:
                        emit_norm_outproj(*pending)
                    pending = (stk_tiles, den_t, qc)
                if pending is not None:
                    emit_norm_outproj(*pending)
                    pending = None
    nc.finalize()
    return nc


_PROG_CACHE = {}


def _get_program(seq=SEQ, repeat=1):
    key = (seq, repeat)
    if key not in _PROG_CACHE:
        _PROG_CACHE[key] = build_program(seq, repeat)
    return _PROG_CACHE[key]


def shard_inputs(q, k, v, Wq, Wk, Wv, Wo, seq=SEQ):
    """Build the 8 per-core input maps (host-side layout prep)."""
    scale = np.float32(QK_SCALE / math.sqrt(B))
    in_maps = []
    for c in range(N_CORES):
        b = c // 4
        hg = c % 4
        heads = [4 * hg + j for j in range(HPC)]
        wq_s = np.concatenate([Wq[:, h::NUM_HEADS] for h in heads], axis=1) * scale
        wk_s = np.concatenate([Wk[:, h::NUM_HEADS] for h in heads], axis=1)
        wv_s = np.zeros((E, HPC, HD + 1), dtype=np.float32)
        for j, h in enumerate(heads):
            if j % 2 == 0:
                wv_s[:, j, 0:HD] = Wv[:, h::NUM_HEADS]
            else:
                wv_s[:, j, 1:HD + 1] = Wv[:, h::NUM_HEADS]
        wo_s = np.concatenate([Wo[h::NUM_HEADS, :] for h in heads], axis=0)
        ind = np.zeros((HPC, HPC // 2, P), dtype=np.float32)
        for kk in range(HPC):
            for pair in range(HPC // 2):
                for m in range(P):
                    if kk == 2 * pair + m // HD:
                        ind[kk, pair, m] = 1.0
        in_maps.append({
            "xtq": np.ascontiguousarray(q[b][:seq].T),
            "xtk": np.ascontiguousarray(k[b][:seq].T),
            "xtv": np.ascontiguousarray(v[b][:seq].T),
            "wq": np.ascontiguousarray(wq_s),
            "wk": np.ascontiguousarray(wk_s),
            "wv": np.ascontiguousarray(wv_s.reshape(E, HPC * (HD + 1))),
            "wo": np.ascontiguousarray(wo_s),
            "ind": ind,
        })
    return in_maps


def unshard(results, seq=SEQ):
    out = np.zeros((B, seq, E), dtype=np.float32)
    for c in range(N_CORES):
        out[c // 4] += results[c]["out"]
    return out


def kernel(q, k, v, Wq, Wk, Wv, Wo):
    q = np.asarray(q, dtype=np.float32)
    k = np.asarray(k, dtype=np.float32)
    v = np.asarray(v, dtype=np.float32)
    Wq = np.asarray(Wq, dtype=np.float32)
    Wk = np.asarray(Wk, dtype=np.float32)
    Wv = np.asarray(Wv, dtype=np.float32)
    Wo = np.asarray(Wo, dtype=np.float32)
    nc = _get_program()
    in_maps = shard_inputs(q, k, v, Wq, Wk, Wv, Wo)
    res = run_bass_kernel_spmd(nc, in_maps, list(range(N_CORES)))
    return unshard(res.results)



# revision 2
# speedup vs baseline: 3.3234x; 3.3234x over previous
"""Trainium2 Bass kernel for nn_AttnDecoder_87230785782556.

Multi-head attention decoder layer: out = softmax((xq Wq)(xk Wk)^T * s) (xv Wv) Wo
Sharding: 8 cores = 2 batches x 4 head-groups (4 heads each, tensor-parallel
column split of Wq/Wk/Wv, row split of Wo; partial outputs summed on host).

All matmuls run in float32r (full PE rate, ~1.5e-4 rounding). Scores are
computed transposed [kv, q] so exp output feeds attn@V as the moving operand;
an extra all-ones column in the V weights makes the same matmul emit the
softmax denominator. Normalization uses an indicator-matmul to broadcast
1/den across partitions.
"""
import math
import numpy as np

from concourse import bacc, mybir, tile
from concourse.bass_utils import run_bass_kernel_spmd

# Problem constants (hardcoded per contract)
B = 2
SEQ = 2048
E = 1024
NUM_HEADS = 16
HD = 64
QK_SCALE = 0.125
N_CORES = 8
HPC = 4            # heads per core
P = 128
NQ = 512           # q chunk (matmul moving free dim)

F32 = mybir.dt.float32
F32R = mybir.dt.float32r


def build_program(seq=SEQ, repeat=1):
    """Build the per-core SPMD program. Identical on all 8 cores."""
    nc = bacc.Bacc("TRN2", target_bir_lowering=False, debug=False,
                   num_devices=N_CORES)

    n_qc = seq // NQ            # q chunks
    n_kv = seq // P             # kv tiles of 128
    n_kt = E // P               # embedding contraction tiles
    FPC = HPC * HD              # features per core = 256
    n_m = FPC // P              # feature pair-tiles = 2

    xtq = nc.dram_tensor("xtq", [E, seq], F32R, kind="ExternalInput")
    xtk = nc.dram_tensor("xtk", [E, seq], F32R, kind="ExternalInput")
    xtv = nc.dram_tensor("xtv", [E, seq], F32R, kind="ExternalInput")
    wq = nc.dram_tensor("wq", [E, FPC], F32R, kind="ExternalInput")
    wk = nc.dram_tensor("wk", [E, FPC], F32R, kind="ExternalInput")
    wv = nc.dram_tensor("wv", [E, HPC * (HD + 1)], F32R, kind="ExternalInput")
    wo = nc.dram_tensor("wo", [FPC, E], F32R, kind="ExternalInput")
    ind = nc.dram_tensor("ind", [HPC, n_m, P], F32R, kind="ExternalInput")
    out = nc.dram_tensor("out", [seq, E], F32, kind="ExternalOutput")

    with tile.TileContext(nc) as tc, nc.allow_low_precision("f32r pipeline"):
        import contextlib
        ctx = contextlib.ExitStack()
        with ctx:
            consts = ctx.enter_context(tc.tile_pool(name="consts", bufs=1))
            bigs = ctx.enter_context(tc.tile_pool(name="bigs", bufs=1))
            vhp = ctx.enter_context(tc.tile_pool(name="vhp", bufs=n_kv))
            xs = ctx.enter_context(tc.tile_pool(name="xs", bufs=6))
            expp = ctx.enter_context(tc.tile_pool(name="expp", bufs=11))
            stkp = ctx.enter_context(tc.tile_pool(name="stkp", bufs=4))
            denp = ctx.enter_context(tc.tile_pool(name="denp", bufs=2))
            finp = ctx.enter_context(tc.tile_pool(name="finp", bufs=2))
            ps = ctx.enter_context(tc.tile_pool(name="ps", bufs=4, space="PSUM"))

            # ---- resident constants ----
            wq_t = consts.tile([P, n_kt, FPC], F32R, name="wq_t", tag="wq")
            wk_t = consts.tile([P, n_kt, FPC], F32R, name="wk_t", tag="wk")
            wv_t = consts.tile([P, n_kt, HPC * (HD + 1)], F32R, name="wv_t", tag="wv")
            wo_t = consts.tile([P, n_m, E], F32R, name="wo_t", tag="wo")
            ind_t = consts.tile([HPC, n_m, P], F32R, name="ind_t", tag="ind")
            nc.sync.dma_start(out=wq_t, in_=wq.ap().rearrange("(t p) m -> p t m", p=P))
            nc.sync.dma_start(out=wk_t, in_=wk.ap().rearrange("(t p) m -> p t m", p=P))
            nc.sync.dma_start(out=wv_t, in_=wv.ap().rearrange("(t p) m -> p t m", p=P))
            nc.sync.dma_start(out=wo_t, in_=wo.ap().rearrange("(t p) m -> p t m", p=P))
            nc.sync.dma_start(out=ind_t, in_=ind.ap())

            import contextlib as _ctxlib

            rep_cm = tc.For_i(0, repeat) if repeat > 1 else _ctxlib.nullcontext()
            with rep_cm:
                # ---- K projection: khT[m] [128, seq], partitions = pair feats ----
                khT = [bigs.tile([P, seq], F32R, name=f"khT{m}", tag=f"khT{m}") for m in range(n_m)]
                qhT = [bigs.tile([P, seq], F32R, name=f"qhT{m}", tag=f"qhT{m}") for m in range(n_m)]
                for nq in range(n_qc):
                    ps_t = ps.tile([P, 2, NQ], F32, name="ps_t", tag="ps")
                    for kt in range(n_kt):
                        x_t = xs.tile([P, NQ], F32R, name="xk_t", tag="x")
                        nc.sync.dma_start(
                            out=x_t,
                            in_=xtk.ap()[P * kt:P * (kt + 1), NQ * nq:NQ * (nq + 1)])
                        for m in range(n_m):
                            nc.tensor.matmul(
                                ps_t[:, m, :],
                                wk_t[:, kt, P * m:P * (m + 1)],
                                x_t,
                                start=(kt == 0), stop=(kt == n_kt - 1))
                    for m in range(n_m):
                        nc.vector.tensor_copy(
                            khT[m][:, NQ * nq:NQ * (nq + 1)], ps_t[:, m, :])

                # ---- V projection: vh tiles [128, 4, 65] (65th col -> ones) ----
                vh_tiles = [vhp.tile([P, HPC, HD + 1], F32R, name=f"vh{i}", tag="vh")
                            for i in range(n_kv)]
                for mb in range(n_kv // 4):
                    ps_v = [ps.tile([P, 2, NQ], F32, name="ps_t", tag="ps") for _ in range(2)]
                    for kt in range(n_kt):
                        xv_t = xs.tile([P, NQ], F32R, name="xv_t", tag="x")
                        nc.sync.dma_start(
                            out=xv_t,
                            in_=xtv.ap()[P * kt:P * (kt + 1), NQ * mb:NQ * (mb + 1)])
                        for sub in range(4):
                            nc.tensor.matmul(
                                ps_v[sub // 2][:, sub % 2, 0:HPC * (HD + 1)],
                                xv_t[:, P * sub:P * (sub + 1)],
                                wv_t[:, kt, :],
                                start=(kt == 0), stop=(kt == n_kt - 1))
                    for sub in range(4):
                        mk = 4 * mb + sub
                        nc.vector.tensor_copy(
                            vh_tiles[mk],
                            ps_v[sub // 2][:, sub % 2, 0:HPC * (HD + 1)].rearrange(
                                "p (h c) -> p h c", h=HPC))
                        # ones slots (zero in wv): even heads col 64, odd heads col 0
                        nc.vector.tensor_scalar_add(
                            vh_tiles[mk][:, 0::2, HD], vh_tiles[mk][:, 0::2, HD], 1.0)
                        nc.vector.tensor_scalar_add(
                            vh_tiles[mk][:, 1::2, 0], vh_tiles[mk][:, 1::2, 0], 1.0)

                # ---- Q projection (per chunk) + attention + output projection ----
                def emit_norm_outproj(stk_tiles, den_t, qc):
                    # normalize: bcast 1/den across partitions via indicator matmul
                    rcp_t = denp.tile([HPC, NQ], F32R, name="rcp_t", tag="rcp")
                    nc.vector.reciprocal(rcp_t, den_t)
                    for pair in range(n_m):
                        bc_ps = ps.tile([P, 2, NQ], F32, name="ps_t", tag="ps")
                        nc.tensor.matmul(bc_ps[:, 0, :], ind_t[:, pair, :], rcp_t,
                                         start=True, stop=True)
                        nc.vector.tensor_mul(stk_tiles[pair], stk_tiles[pair],
                                             bc_ps[:, 0, :])
                    # output projection: out[q, E] = sum_pair stk[pair].T @ wo[pair]
                    for qs in range(NQ // P):
                        op_ps = ps.tile([P, 2, NQ], F32, name="ps_t", tag="ps")
                        for nch in range(E // NQ):
                            for pair in range(n_m):
                                nc.tensor.matmul(
                                    op_ps[:, nch, :],
                                    stk_tiles[pair][:, P * qs:P * (qs + 1)],
                                    wo_t[:, pair, NQ * nch:NQ * (nch + 1)],
                                    start=(pair == 0), stop=(pair == n_m - 1))
                        fin_t = finp.tile([P, 2, NQ], F32, name="fin_t", tag="fin")
                        for nch in range(E // NQ):
                            nc.vector.tensor_copy(fin_t[:, nch, :], op_ps[:, nch, :])
                        r0 = NQ * qc + P * qs
                        nc.sync.dma_start(
                            out=out.ap()[r0:r0 + P, :],
                            in_=fin_t.rearrange("p a b -> p (a b)"))

                pending = None
                for qc in range(n_qc):
                    ps_q = ps.tile([P, 2, NQ], F32, name="ps_t", tag="ps")
                    for kt in range(n_kt):
                        xq_t = xs.tile([P, NQ], F32R, name="xq_t", tag="x")
                        nc.sync.dma_start(
                            out=xq_t,
                            in_=xtq.ap()[P * kt:P * (kt + 1), NQ * qc:NQ * (qc + 1)])
                        for m in range(n_m):
                            nc.tensor.matmul(
                                ps_q[:, m, :],
                                wq_t[:, kt, P * m:P * (m + 1)],
                                xq_t,
                                start=(kt == 0), stop=(kt == n_kt - 1))
                    for m in range(n_m):
                        nc.vector.tensor_copy(
                            qhT[m][:, NQ * qc:NQ * (qc + 1)], ps_q[:, m, :])

                    den_t = denp.tile([HPC, NQ], F32R, name="den_t", tag="den")
                    stk_tiles = []
                    for pair in range(n_m):
                        att_ps = ps.tile([P, 2, NQ], F32, name="ps_t", tag="ps")
                        exp_tiles = []

                        def emit_attnv(g, pair=pair, att_ps=att_ps,
                                       exp_tiles=exp_tiles):
                            for s in range(2):
                                kv = 2 * g + s
                                for h01 in range(2):
                                    nc.tensor.matmul(
                                        att_ps[0:HD + 1, h01, :],
                                        vh_tiles[kv][:, 2 * pair + h01, :],
                                        exp_tiles[g][h01][:, s, :],
                                        start=(kv == 0), stop=(kv == n_kv - 1))

                        # software pipeline: attn@V for group g-1 interleaves with
                        # scores+exp for group g, so exp tiles retire promptly and
                        # the PE never queues behind a not-yet-runnable exp.
                        for g in range(n_kv // 2):
                            sc = []
                            for h01 in range(2):
                                sc_t = ps.tile([P, 2, NQ], F32, name="ps_t", tag="ps")
                                for s in range(2):
                                    kv = 2 * g + s
                                    nc.tensor.matmul(
                                        sc_t[:, s, :],
                                        khT[pair][64 * h01:64 * (h01 + 1),
                                                  P * kv:P * (kv + 1)],
                                        qhT[pair][64 * h01:64 * (h01 + 1),
                                                  NQ * qc:NQ * (qc + 1)],
                                        start=True, stop=True,
                                        tile_position=(64 * h01, 0))
                                sc.append(sc_t)
                            pair_exp = []
                            for h01 in range(2):
                                e_t = expp.tile([P, 2, NQ], F32R, name="exp_t", tag="exp")
                                nc.scalar.activation(
                                    e_t, sc[h01],
                                    mybir.ActivationFunctionType.Exp)
                                pair_exp.append(e_t)
                            exp_tiles.append(pair_exp)
                            if g > 0:
                                emit_attnv(g - 1)
                        emit_attnv(n_kv // 2 - 1)
                        # stack the two heads' outputs [feat, q] into one tile.
                        # PSUM is not DMA-accessible: stage through SBUF, then
                        # SBUF->SBUF DMAs do the cross-partition moves.
                        stk_t = stkp.tile([P, NQ], F32R, name="stk_t", tag="stk")
                        nc.vector.tensor_copy(stk_t[0:HD, :], att_ps[0:HD, 0, :])
                        sodd_t = stkp.tile([P, NQ], F32R, name="sodd_t", tag="sodd", bufs=2)
                        nc.vector.tensor_copy(sodd_t[0:HD + 1, :],
                                              att_ps[0:HD + 1, 1, :])
                        dstg_t = stkp.tile([P, NQ], F32R, name="dstg_t", tag="dstg", bufs=2)
                        nc.vector.tensor_copy(dstg_t[HD:HD + 1, :],
                                              att_ps[HD:HD + 1, 0, :])
                        nc.sync.dma_start(out=stk_t[HD:P, :], in_=sodd_t[1:HD + 1, :])
                        # denominators: even head at psum row 64 slot 0, odd at row 0 slot 1
                        nc.sync.dma_start(out=den_t[2 * pair:2 * pair + 1, :],
                                          in_=dstg_t[HD:HD + 1, :])
                        nc.sync.dma_start(out=den_t[2 * pair + 1:2 * pair + 2, :],
                                          in_=sodd_t[0:1, :])
                        stk_tiles.append(stk_t)

                    # defer normalization + output projection by one chunk so the
                    # den DMA/recip chain overlaps the next chunk's attention.
                    if pending is not None:
                        emit_norm_outproj(*pending)
                    pending = (stk_tiles, den_t, qc)
                if pending is not None:
                    emit_norm_outproj(*pending)
                    pending = None
    nc.finalize()
    return nc


_PROG_CACHE = {}


def _get_program(seq=SEQ, repeat=1):
    key = (seq, repeat)
    if key not in _PROG_CACHE:
        _PROG_CACHE[key] = build_program(seq, repeat)
    return _PROG_CACHE[key]


def shard_inputs(q, k, v, Wq, Wk, Wv, Wo, seq=SEQ):
    """Build the 8 per-core input maps (host-side layout prep)."""
    scale = np.float32(QK_SCALE / math.sqrt(B))
    in_maps = []
    for c in range(N_CORES):
        b = c // 4
        hg = c % 4
        heads = [4 * hg + j for j in range(HPC)]
        wq_s = np.concatenate([Wq[:, h::NUM_HEADS] for h in heads], axis=1) * scale
        wk_s = np.concatenate([Wk[:, h::NUM_HEADS] for h in heads], axis=1)
        wv_s = np.zeros((E, HPC, HD + 1), dtype=np.float32)
        for j, h in enumerate(heads):
            if j % 2 == 0:
                wv_s[:, j, 0:HD] = Wv[:, h::NUM_HEADS]
            else:
                wv_s[:, j, 1:HD + 1] = Wv[:, h::NUM_HEADS]
        wo_s = np.concatenate([Wo[h::NUM_HEADS, :] for h in heads], axis=0)
        ind = np.zeros((HPC, HPC // 2, P), dtype=np.float32)
        for kk in range(HPC):
            for pair in range(HPC // 2):
                for m in range(P):
                    if kk == 2 * pair + m // HD:
                        ind[kk, pair, m] = 1.0
        in_maps.append({
            "xtq": np.ascontiguousarray(q[b][:seq].T),
            "xtk": np.ascontiguousarray(k[b][:seq].T),
            "xtv": np.ascontiguousarray(v[b][:seq].T),
            "wq": np.ascontiguousarray(wq_s),
            "wk": np.ascontiguousarray(wk_s),
            "wv": np.ascontiguousarray(wv_s.reshape(E, HPC * (HD + 1))),
            "wo": np.ascontiguousarray(wo_s),
            "ind": ind,
        })
    return in_maps


def unshard(results, seq=SEQ):
    out = np.zeros((B, seq, E), dtype=np.float32)
    for c in range(N_CORES):
        out[c // 4] += results[c]["out"]
    return out


def kernel(q, k, v, Wq, Wk, Wv, Wo):
    q = np.asarray(q, dtype=np.float32)
    k = np.asarray(k, dtype=np.float32)
    v = np.asarray(v, dtype=np.float32)
    Wq = np.asarray(Wq, dtype=np.float32)
    Wk = np.asarray(Wk, dtype=np.float32)
    Wv = np.asarray(Wv, dtype=np.float32)
    Wo = np.asarray(Wo, dtype=np.float32)
    nc = _get_program()
    in_maps = shard_inputs(q, k, v, Wq, Wk, Wv, Wo)
    res = run_bass_kernel_spmd(nc, in_maps, list(range(N_CORES)))
    return unshard(res.results)



# revision 19
# speedup vs baseline: 4.6252x; 1.3917x over previous
"""Trainium2 Bass kernel for nn_AttnDecoder_87230785782556.

Multi-head attention decoder layer: out = softmax((xq Wq)(xk Wk)^T * s) (xv Wv) Wo
Sharding: 8 cores = 2 batches x 4 head-groups (4 heads each, tensor-parallel
column split of Wq/Wk/Wv, row split of Wo; partial outputs summed on host).

Inputs and weights are cast to bf16 on the host; projections run bf16 ->
f32 PSUM. Scores are f32r [kv, q]; exp outputs bf16 tiles that feed attn@V
as the 128x128 *stationary* operand with bf16 V tiles moving (65 rows), so
attn@V uses all 128 output partitions (q). The V weights carry an extra
all-ones column, so column 64 (even heads) / 0 (odd heads) of the attn@V
output is the softmax denominator, normalized per-q-partition on DVE and
transposed back to [feat, q] by PE transposes straight into PSUM.

Schedule (streaming): only K/Q/V for the first 512-column block run up
front; the remaining K/V projection blocks, next-chunk xq prefetch + Q
projection, previous chunk's output projection, and the deferred transpose
epilogues are all "dripped" into the attention g-loops at fixed slots, so
PE/ACT/DVE/DMA stay concurrently busy from ~10us onward. Drip units
allocate and retire their PSUM tiles atomically to keep the in-order
engine queues deadlock-free.
"""
import math
import numpy as np

from concourse import bacc, mybir, tile
from concourse.bass_utils import run_bass_kernel_spmd

# Problem constants (hardcoded per contract)
B = 2
SEQ = 2048
E = 1024
NUM_HEADS = 16
HD = 64
QK_SCALE = 0.125
N_CORES = 8
HPC = 4            # heads per core
P = 128
NQ = 512           # q chunk (matmul moving free dim)

F32 = mybir.dt.float32
F32R = mybir.dt.float32r
BF16 = mybir.dt.bfloat16


def build_program(seq=SEQ, repeat=1):
    """Build the per-core SPMD program. Identical on all 8 cores."""
    nc = bacc.Bacc("TRN2", target_bir_lowering=False, debug=False,
                   num_devices=N_CORES)

    n_qc = seq // NQ            # q chunks
    n_kv = seq // P             # kv tiles of 128
    n_kt = E // P               # embedding contraction tiles
    FPC = HPC * HD              # features per core = 256
    n_m = FPC // P              # feature pair-tiles = 2
    n_qs = NQ // P              # q subtiles per chunk = 4

    xtq = nc.dram_tensor("xtq", [E, seq], BF16, kind="ExternalInput")
    xtk = nc.dram_tensor("xtk", [E, seq], BF16, kind="ExternalInput")
    xtv = nc.dram_tensor("xtv", [E, seq], BF16, kind="ExternalInput")
    wq = nc.dram_tensor("wq", [E, FPC], BF16, kind="ExternalInput")
    wk = nc.dram_tensor("wk", [E, FPC], BF16, kind="ExternalInput")
    wv = nc.dram_tensor("wv", [E, HPC * (HD + 1)], BF16, kind="ExternalInput")
    wo = nc.dram_tensor("wo", [FPC, E], BF16, kind="ExternalInput")
    idn = nc.dram_tensor("idn", [P, P], BF16, kind="ExternalInput")
    out = nc.dram_tensor("out", [seq, E], F32, kind="ExternalOutput")

    with tile.TileContext(nc) as tc, nc.allow_low_precision("bf16/f32r pipeline"):
        import contextlib
        ctx = contextlib.ExitStack()
        with ctx:
            consts = ctx.enter_context(tc.tile_pool(name="consts", bufs=1))
            bigs = ctx.enter_context(tc.tile_pool(name="bigs", bufs=1))
            vhp = ctx.enter_context(tc.tile_pool(name="vhp", bufs=n_kv))
            xkp = ctx.enter_context(tc.tile_pool(name="xkp", bufs=n_kt))
            xvp = ctx.enter_context(tc.tile_pool(name="xvp", bufs=n_kt))
            xqp = ctx.enter_context(tc.tile_pool(name="xqp", bufs=n_kt))
            expp = ctx.enter_context(tc.tile_pool(name="expp", bufs=34))
            avnp = ctx.enter_context(tc.tile_pool(name="avnp", bufs=3))
            rcpp = ctx.enter_context(tc.tile_pool(name="rcpp", bufs=2))
            stkp = ctx.enter_context(tc.tile_pool(name="stkp", bufs=4))
            finp = ctx.enter_context(tc.tile_pool(name="finp", bufs=2))
            ps = ctx.enter_context(tc.tile_pool(name="ps", bufs=4, space="PSUM"))

            # ---- resident constants (wk first: K proj block 0 runs first) ----
            wk_t = consts.tile([P, n_kt, FPC], BF16, name="wk_t", tag="wk")
            wq_t = consts.tile([P, n_kt, FPC], BF16, name="wq_t", tag="wq")
            wv_t = consts.tile([P, n_kt, HPC * (HD + 1)], BF16, name="wv_t", tag="wv")
            wo_t = consts.tile([P, n_m, E], BF16, name="wo_t", tag="wo")
            idn_t = consts.tile([P, P], BF16, name="idn_t", tag="idn")
            nc.sync.dma_start(out=wk_t, in_=wk.ap().rearrange("(t p) m -> p t m", p=P))
            nc.sync.dma_start(out=wq_t, in_=wq.ap().rearrange("(t p) m -> p t m", p=P))
            nc.sync.dma_start(out=wv_t, in_=wv.ap().rearrange("(t p) m -> p t m", p=P))
            nc.sync.dma_start(out=wo_t, in_=wo.ap().rearrange("(t p) m -> p t m", p=P))
            nc.sync.dma_start(out=idn_t, in_=idn.ap())

            import contextlib as _ctxlib
            rep_cm = tc.For_i(0, repeat) if repeat > 1 else _ctxlib.nullcontext()
            with rep_cm:
                khT = [bigs.tile([P, seq], F32R, name=f"khT{m}", tag=f"khT{m}") for m in range(n_m)]
                qhT = [bigs.tile([P, seq], F32R, name=f"qhT{m}", tag=f"qhT{m}") for m in range(n_m)]
                vh_tiles = [vhp.tile([P, HPC, HD + 1], BF16, name=f"vh{i}", tag="vh")
                            for i in range(n_kv)]

                def loads(pool, src, blk, tagname):
                    tiles = []
                    for kt in range(n_kt):
                        t = pool.tile([P, NQ], BF16, name=f"{tagname}_t", tag=tagname)
                        nc.sync.dma_start(
                            out=t,
                            in_=src.ap()[P * kt:P * (kt + 1), NQ * blk:NQ * (blk + 1)])
                        tiles.append(t)
                    return tiles

                def kproj_mm(blk, x_tiles):
                    ps_t = ps.tile([P, 2, NQ], F32, name="ps_t", tag="ps")
                    for kt in range(n_kt):
                        for m in range(n_m):
                            nc.tensor.matmul(
                                ps_t[:, m, :],
                                wk_t[:, kt, P * m:P * (m + 1)],
                                x_tiles[kt],
                                start=(kt == 0), stop=(kt == n_kt - 1))
                    for m in range(n_m):
                        nc.vector.tensor_copy(
                            khT[m][:, NQ * blk:NQ * (blk + 1)], ps_t[:, m, :])

                def qproj_mm(qc, x_tiles):
                    ps_q = ps.tile([P, 2, NQ], F32, name="ps_t", tag="ps")
                    for kt in range(n_kt):
                        for m in range(n_m):
                            nc.tensor.matmul(
                                ps_q[:, m, :],
                                wq_t[:, kt, P * m:P * (m + 1)],
                                x_tiles[kt],
                                start=(kt == 0), stop=(kt == n_kt - 1))
                    for m in range(n_m):
                        nc.vector.tensor_copy(
                            qhT[m][:, NQ * qc:NQ * (qc + 1)], ps_q[:, m, :])

                def vproj_mm(blk, x_tiles):
                    ps_v = [ps.tile([P, 2, NQ], F32, name="ps_t", tag="ps")
                            for _ in range(2)]
                    for kt in range(n_kt):
                        for sub in range(4):
                            nc.tensor.matmul(
                                ps_v[sub // 2][:, sub % 2, 0:HPC * (HD + 1)],
                                x_tiles[kt][:, P * sub:P * (sub + 1)],
                                wv_t[:, kt, :],
                                start=(kt == 0), stop=(kt == n_kt - 1))
                    for sub in range(4):
                        mk = 4 * blk + sub
                        nc.vector.tensor_copy(
                            vh_tiles[mk],
                            ps_v[sub // 2][:, sub % 2, 0:HPC * (HD + 1)].rearrange(
                                "p (h c) -> p h c", h=HPC))
                        # ones slots (zero in wv): even heads col 64, odd heads col 0
                        nc.vector.tensor_scalar_add(
                            vh_tiles[mk][:, 0::2, HD], vh_tiles[mk][:, 0::2, HD], 1.0)
                        nc.vector.tensor_scalar_add(
                            vh_tiles[mk][:, 1::2, 0], vh_tiles[mk][:, 1::2, 0], 1.0)

                def outproj_qs_unit(stk_tiles, qc, qs):
                    op_ps = ps.tile([P, 2, NQ], F32, name="ps_t", tag="ps")
                    for nch in range(E // NQ):
                        for pair in range(n_m):
                            nc.tensor.matmul(
                                op_ps[:, nch, :],
                                stk_tiles[pair][:, P * qs:P * (qs + 1)],
                                wo_t[:, pair, NQ * nch:NQ * (nch + 1)],
                                start=(pair == 0), stop=(pair == n_m - 1))
                    fin_t = finp.tile([P, 2, NQ], F32, name="fin_t", tag="fin")
                    for nch in range(E // NQ):
                        nc.vector.tensor_copy(fin_t[:, nch, :], op_ps[:, nch, :])
                    r0 = NQ * qc + P * qs
                    nc.sync.dma_start(
                        out=out.ap()[r0:r0 + P, :],
                        in_=fin_t.rearrange("p a b -> p (a b)"))

                # ---- prologue: block 0 of K, Q(chunk 0), V + block-1 loads ----
                kproj_mm(0, loads(xkp, xtk, 0, "xk"))
                qproj_mm(0, loads(xqp, xtq, 0, "xq"))
                vproj_mm(0, loads(xvp, xtv, 0, "xv"))
                kx = {1: loads(xkp, xtk, 1, "xk")}
                vx = {1: loads(xvp, xtv, 1, "xv")}

                # ---- attention sweeps ----
                # attn@V for one (pair, qs): complete accumulation sweep over
                # all kv tiles into av [q, h01, feat] with the two h01 groups
                # in SEPARATE PSUM banks (only one open accumulation group per
                # bank is allowed), then per-q-partition normalize on DVE.
                # The PE transposes back to [feat, q] are deferred one sweep
                # so they never wait on the DVE chain.
                def make_pair_state(pair, exps, stk_t):
                    state = {"prev": None}

                    def sweep(qs):
                        av = ps.tile([P, 2, NQ], F32, name="av", tag="ps")
                        for kv in range(n_kv):
                            g, s = kv // 2, kv % 2
                            for h01 in range(2):
                                nc.tensor.matmul(
                                    av[:, h01, 0:HD + 1],
                                    exps[g][h01][:, s, P * qs:P * (qs + 1)],
                                    vh_tiles[kv][:, 2 * pair + h01, :],
                                    start=(kv == 0), stop=(kv == n_kv - 1))
                        rcp_t = rcpp.tile([P, 2, 1], F32R, name="rcp_t", tag="rcp")
                        nc.vector.reciprocal(rcp_t[:, 0, :], av[:, 0, HD:HD + 1])
                        nc.vector.reciprocal(rcp_t[:, 1, :], av[:, 1, 0:1])
                        avn = avnp.tile([P, 2, HD], BF16, name="avn", tag="avn")
                        nc.vector.tensor_mul(
                            avn[:, 0], av[:, 0, 0:HD],
                            rcp_t[:, 0].broadcast_to([P, HD]))
                        nc.vector.tensor_mul(
                            avn[:, 1], av[:, 1, 1:HD + 1],
                            rcp_t[:, 1].broadcast_to([P, HD]))
                        prev = state["prev"]
                        state["prev"] = (avn, qs)
                        if prev is not None:
                            flush_prev(prev)

                    def flush_prev(prev):
                        avn, qs = prev
                        tr = ps.tile([P, P], BF16, name="tr", tag="ps")
                        for h01 in range(2):
                            nc.tensor.transpose(
                                tr[64 * h01:64 * (h01 + 1), :],
                                avn[:, h01, :], idn_t)
                        nc.vector.tensor_copy(stk_t[:, P * qs:P * (qs + 1)], tr)

                    def finish():
                        flush_prev(state["prev"])
                        state["prev"] = None

                    return sweep, finish

                # ---- attention chunks ----
                pending = None          # (stk_tiles, qc) of finished chunk
                next_xq = None          # prefetched xq tiles for chunk qc+1
                carry = []              # (slot, thunk) deferred into next chunk
                for qc in range(n_qc):
                    drip = {}

                    def at(s, thunk):
                        drip.setdefault(s, []).append(thunk)

                    for s, thunk in carry:
                        at(s, thunk)
                    carry = []
                    if qc == 0:
                        # stream remaining K/V projection blocks: B(b) compute
                        # at slots 2b-2 / 2b-1, next block's loads alongside
                        for b in (1, 2, 3):
                            at(2 * b - 2, lambda b=b: kproj_mm(b, kx.pop(b)))
                            if b + 1 <= 3:
                                at(2 * b - 2, lambda b=b: kx.__setitem__(
                                    b + 1, loads(xkp, xtk, b + 1, "xk")))
                            at(2 * b - 1, lambda b=b: vproj_mm(b, vx.pop(b)))
                            if b + 1 <= 3:
                                at(2 * b - 1, lambda b=b: vx.__setitem__(
                                    b + 1, loads(xvp, xtv, b + 1, "xv")))
                    if pending is not None:
                        pstk, pqc = pending
                        for qs in range(n_qs):
                            at(9 + 2 * qs, lambda pstk=pstk, pqc=pqc, qs=qs:
                               outproj_qs_unit(pstk, pqc, qs))
                    if qc + 1 < n_qc:
                        acc = []
                        for s, kts in ((1, (0, 1, 2, 3)), (3, (4, 5, 6, 7))):
                            def xq_load(qc1=qc + 1, kts=kts, acc=acc):
                                for kt in kts:
                                    t = xqp.tile([P, NQ], BF16, name="xq_t", tag="xq")
                                    nc.sync.dma_start(
                                        out=t,
                                        in_=xtq.ap()[P * kt:P * (kt + 1),
                                                     NQ * qc1:NQ * (qc1 + 1)])
                                    acc.append(t)
                            at(s, xq_load)
                        at(12, lambda qc1=qc + 1, acc=acc: qproj_mm(qc1, acc))
                        next_xq = acc

                    stk_tiles = []
                    slot = 0
                    for pair in range(n_m):
                        exps = []
                        stk_t = stkp.tile([P, NQ], BF16, name="stk_t", tag="stk")
                        sweep, finish = make_pair_state(pair, exps, stk_t)
                        # pair 0 sweeps in pair 1's g-slots; pair 1 sweeps in
                        # the next chunk's pair-0 g-slots (or the tail)
                        if pair == 0:
                            for i in range(n_qs):
                                at(8 + 2 * i, lambda i=i, sweep=sweep: sweep(i))
                            at(15, finish)
                        else:
                            for i in range(n_qs):
                                carry.append((2 * i, lambda i=i, sweep=sweep: sweep(i)))
                            carry.append((7, finish))
                        stk_tiles.append(stk_t)

                        for g in range(n_kv // 2):
                            if g == 0:
                                for thunk in drip.pop(slot, []):
                                    thunk()
                            sc = []
                            for h01 in range(2):
                                sc_t = ps.tile([P, 2, NQ], F32, name="ps_t", tag="ps")
                                for s in range(2):
                                    kv = 2 * g + s
                                    nc.tensor.matmul(
                                        sc_t[:, s, :],
                                        khT[pair][64 * h01:64 * (h01 + 1),
                                                  P * kv:P * (kv + 1)],
                                        qhT[pair][64 * h01:64 * (h01 + 1),
                                                  NQ * qc:NQ * (qc + 1)],
                                        start=True, stop=True,
                                        tile_position=(64 * h01, 0))
                                sc.append(sc_t)
                            pair_exp = []
                            for h01 in range(2):
                                e_t = expp.tile([P, 2, NQ], BF16, name="exp_t", tag="exp")
                                nc.scalar.activation(
                                    e_t, sc[h01],
                                    mybir.ActivationFunctionType.Exp)
                                pair_exp.append(e_t)
                            exps.append(pair_exp)
                            if g > 0:
                                for thunk in drip.pop(slot, []):
                                    thunk()
                            slot += 1

                    for s in sorted(drip):
                        for thunk in drip[s]:
                            thunk()
                    pending = (stk_tiles, qc)

                # tail: last chunk's pair-1 sweeps, then its output projection
                for s, thunk in sorted(carry, key=lambda x: x[0]):
                    thunk()
                carry = []
                if pending is not None:
                    pstk, pqc = pending
                    for qs in range(n_qs):
                        outproj_qs_unit(pstk, pqc, qs)
                    pending = None
    nc.finalize()
    return nc


_PROG_CACHE = {}


def _get_program(seq=SEQ, repeat=1):
    key = (seq, repeat)
    if key not in _PROG_CACHE:
        _PROG_CACHE[key] = build_program(seq, repeat)
    return _PROG_CACHE[key]


def shard_inputs(q, k, v, Wq, Wk, Wv, Wo, seq=SEQ):
    """Build the 8 per-core input maps (host-side layout prep)."""
    import ml_dtypes
    bf16 = ml_dtypes.bfloat16
    scale = np.float32(QK_SCALE / math.sqrt(B))
    idn = np.eye(P, dtype=np.float32).astype(bf16)
    in_maps = []
    for c in range(N_CORES):
        b = c // 4
        hg = c % 4
        heads = [4 * hg + j for j in range(HPC)]
        wq_s = np.concatenate([Wq[:, h::NUM_HEADS] for h in heads], axis=1) * scale
        wk_s = np.concatenate([Wk[:, h::NUM_HEADS] for h in heads], axis=1)
        wv_s = np.zeros((E, HPC, HD + 1), dtype=np.float32)
        for j, h in enumerate(heads):
            if j % 2 == 0:
                wv_s[:, j, 0:HD] = Wv[:, h::NUM_HEADS]
            else:
                wv_s[:, j, 1:HD + 1] = Wv[:, h::NUM_HEADS]
        wo_s = np.concatenate([Wo[h::NUM_HEADS, :] for h in heads], axis=0)
        in_maps.append({
            "xtq": np.ascontiguousarray(q[b][:seq].T).astype(bf16),
            "xtk": np.ascontiguousarray(k[b][:seq].T).astype(bf16),
            "xtv": np.ascontiguousarray(v[b][:seq].T).astype(bf16),
            "wq": np.ascontiguousarray(wq_s).astype(bf16),
            "wk": np.ascontiguousarray(wk_s).astype(bf16),
            "wv": np.ascontiguousarray(wv_s.reshape(E, HPC * (HD + 1))).astype(bf16),
            "wo": np.ascontiguousarray(wo_s).astype(bf16),
            "idn": idn,
        })
    return in_maps


def unshard(results, seq=SEQ):
    out = np.zeros((B, seq, E), dtype=np.float32)
    for c in range(N_CORES):
        out[c // 4] += results[c]["out"]
    return out


def kernel(q, k, v, Wq, Wk, Wv, Wo):
    q = np.asarray(q, dtype=np.float32)
    k = np.asarray(k, dtype=np.float32)
    v = np.asarray(v, dtype=np.float32)
    Wq = np.asarray(Wq, dtype=np.float32)
    Wk = np.asarray(Wk, dtype=np.float32)
    Wv = np.asarray(Wv, dtype=np.float32)
    Wo = np.asarray(Wo, dtype=np.float32)
    nc = _get_program()
    in_maps = shard_inputs(q, k, v, Wq, Wk, Wv, Wo)
    res = run_bass_kernel_spmd(nc, in_maps, list(range(N_CORES)))
    return unshard(res.results)
